# revision 1
# baseline (speedup 1.0000x reference)
"""AdaAttN + FISTA sparse-coding Trainium2 kernel (8 NeuronCores, SPMD).

Sharding: core i handles batch b = i//4.  The attention stage is q-sharded
(4 q-blocks of 576 per batch); an AllGather inside each 4-core batch group
exchanges the attention stats (mean/std), and each core selects its
128-channel slice with a host-supplied one-hot mask (keeps the SPMD graph
rank-independent).  Each core then runs projections + group-norm + FISTA
for its slice and the host concatenates the 8 channel shards.

Math notes (validated against the reference on host):
  * Attention stats via unnormalized E = exp(logits) plus a denominator
    column; no row-max subtraction (max |logit| ~ 31, safe in fp32/bf16).
  * FISTA gradient uses WtW = A^T A:  arg = y - y@(lr*WtW) + lr*x@A.
    Keeping the identity exact in fp32 and only the small lr*WtW matrix
    in fp16 gives ~1.2e-3 relative error end-to-end.
  * The reference's early-exit tolerance never triggers in 43 iters
    (verified numerically), so the loop is unconditional.
All matmuls use fp16 operands (bf16 for the exp/stats path for range)
with fp32 PSUM accumulation.
"""

import math
from contextlib import ExitStack

import numpy as np

B, C, Hs, Ws = 2, 512, 48, 48
HW = Hs * Ws          # 2304
M = HW
GROUPS, GS = 256, 9
ALPHA = 0.05
MAXITER = 43
EPS = 1e-5
POWER_ITERS = 100

NCORES = 8
QB = HW // 4          # 576 q per attention core
HC = HW // 128        # 18 h-chunks
CC = C // 128         # 4 c-chunks
QCH = [128, 128, 128, 128, 64]     # q-chunks within a 576 q-block
SPANS = [(0, 512), (512, 512), (1024, 512), (1536, 512), (2048, 256)]

_CACHE = {}



def _patch_tile():
    """Walrus in this environment rejects instructions carrying more than one
    embedded semaphore wait ("Too many sync wait commands").  Split every
    multi-wait instruction Tile emits into single-wait NOPs ahead of it —
    the exact encoding raw-bass wait_ge uses, which compiles fine."""
    import concourse.tile as tile
    from concourse import mybir
    from concourse.vector_clock import ScopedClock

    if getattr(tile.TileContext, "_wait_split_patched", False):
        return
    orig_commit = tile.TileContext._commit_instruction

    def commit(self, inst, lazy_reg_writes=True):
        si = getattr(inst, "sync_info", None)
        try:
            ok = (si is not None and si.on_wait and len(si.on_wait) > 1
                  and isinstance(inst, mybir.Instruction)
                  and inst.engine != mybir.EngineType.Unassigned)
        except Exception:
            ok = False
        if ok:
            waits = list(si.on_wait)
            for w in waits:
                nop = mybir.InstNoOp(
                    name=self.nc.get_next_instruction_name(),
                    engine=inst.engine,
                    sync_info=mybir.SyncInfo(on_wait=[w], on_update=[]),
                    bass_nofuse=True,
                )
                orig_commit(self, nop, lazy_reg_writes=False)
            inst.sync_info = mybir.SyncInfo(on_wait=[], on_update=list(si.on_update or []))
        return orig_commit(self, inst, lazy_reg_writes)

    def drain_and_barrier(self, tick_clock, wait_clock):
        drain_bi = self.nc.sync.drain()
        di = drain_bi.ins
        wait_clock.add_sem_waits(di, ScopedClock({None: tick_clock.global_clock}))
        si = di.sync_info
        if si is not None and si.on_wait and len(si.on_wait) > 1:
            waits = list(si.on_wait)
            di.sync_info = mybir.SyncInfo(on_wait=[waits[0]],
                                          on_update=list(si.on_update or []))
            for w in waits[1:]:
                nop = mybir.InstNoOp(
                    name=self.nc.get_next_instruction_name(),
                    engine=mybir.EngineType.SP,
                    sync_info=mybir.SyncInfo(on_wait=[w], on_update=[]),
                    bass_nofuse=True,
                )
                self.nc.sync.add_instruction(nop)
        self.nc.all_engine_barrier()
        assert self.sems is not None
        popped = self.nc._tile_sem_poison_stack.pop()
        assert popped is self._sem_poison
        self.nc.clear_and_free_semaphores(list(self.sems.allocated().values()))
        self.nc.all_engine_barrier()

    tile.TileContext._commit_instruction = commit
    tile.TileContext._drain_and_barrier = drain_and_barrier
    tile.TileContext._wait_split_patched = True


def _build_graph(lam, betas, n_iter, debug=False, reps=1):
    _patch_tile()
    import concourse.bass as bass
    import concourse.tile as tile
    from concourse import mybir
    from concourse.masks import make_identity

    f32 = mybir.dt.float32
    f16 = mybir.dt.float16
    bf16 = mybir.dt.bfloat16
    AX = mybir.AxisListType
    OP = mybir.AluOpType
    ACT = mybir.ActivationFunctionType

    nc = bass.Bass(trn_type="TRN2", num_devices=NCORES, debug=False)

    # ---- I/O ----------------------------------------------------------
    din = lambda name, shape, dt: nc.dram_tensor(name, shape, dt, kind="ExternalInput")
    w16d = din("w16", [128, HC * HW], f16)          # lr*WtW, chunk-major
    at16d = din("at16", [HC, 128, HW], f16)         # A^T   [h, m] chunks
    alr16d = din("alr16", [HC, 128, HW], f16)       # lr*A  [m, h] chunks
    stkd = din("stk", [128, CC * HW], f16)          # style_key[b]
    std_ = din("st", [128, CC * HW], f16)           # style[b]
    ckd = din("ck", [128, CC * QB], f16)            # content_key[b][:, qblk]
    contd = din("cont", [128, HW], f16)             # content[b][c-slice]
    fwtd = din("fwt", [128, CC * C], f16)
    gwtd = din("gwt", [128, CC * C], f16)
    hwtd = din("hwt", [128, CC * C], f16)
    fbd = din("fb", [128, CC], f32)
    gbd = din("gb", [128, CC], f32)
    hbbd = din("hbb", [1, C], f16)
    wseld = din("wsel", [128, 4], f32)
    zoutd = nc.dram_tensor("zout", [128, HW], f32, kind="ExternalOutput")
    if debug:
        dmstd = nc.dram_tensor("dbg_mst", [128, 5 * 1024], f16, kind="ExternalOutput")
        dxd = nc.dram_tensor("dbg_x", [128, HW], f16, kind="ExternalOutput")
        dDd = nc.dram_tensor("dbg_D", [128, HW], f32, kind="ExternalOutput")

    ag_in = nc.dram_tensor("ag_in", [QB, 1024], f16)
    ag_out = nc.dram_tensor("ag_out", [4 * QB, 1024], f16)

    with tile.TileContext(nc) as tc:
        _orig_alloc = tc.alloc_tile_pool
        _rep = [0]
        def _alloc(*, name, **kw):
            return _orig_alloc(name=f"{name}_r{_rep[0]}", **kw)
        tc.alloc_tile_pool = _alloc
        for _r in range(reps):
            _rep[0] = _r
            # ---- long-lived pools on the RIGHT stack ----------------------
            p0 = tc.alloc_tile_pool(name="p0", bufs=1, side="right")
            pc_pool = tc.alloc_tile_pool(name="pcp", bufs=1, side="right")
            stream = tc.alloc_tile_pool(name="stream", bufs=6, side="right")
            ident = p0.tile([128, 128], f16, tag="ident")
            make_identity(nc, ident)
            ident32 = p0.tile([128, 128], f32, tag="ident32")
            make_identity(nc, ident32)
            ones_b = p0.tile([128, 1], bf16, tag="ones")
            nc.vector.memset(ones_b, 1.0)
            wsel = p0.tile([128, 4], f32, tag="wsel")
            nc.sync.dma_start(out=wsel, in_=wseld[:, :])
            lamn = p0.tile([128, 1], f32, tag="lamn")
            nc.vector.memset(lamn, -float(lam))

            # ================= Phase A: attention stats ====================
            pam = tc.alloc_tile_pool(name="pam", bufs=1)
            pab = tc.alloc_tile_pool(name="pab", bufs=2)
            pah = tc.alloc_tile_pool(name="pah", bufs=1)
            pag = tc.alloc_tile_pool(name="pag", bufs=1)
            paw = tc.alloc_tile_pool(name="paw", bufs=2)
            pconv = tc.alloc_tile_pool(name="pconv", bufs=2, space="PSUM")

            mst = pam.tile([128, 5, 1024], f16, tag="mst")

            hwt = paw.tile([128, CC * C], f16, tag="wconv")
            nc.sync.dma_start(out=hwt, in_=hwtd[:, :])
            hbb = paw.tile([1, C], f16, tag="hbb")
            nc.sync.dma_start(out=hbb, in_=hbbd[:, :])
            ones16 = p0.tile([1, 128], f16, tag="ones16")
            nc.vector.memset(ones16, 1.0)
            st_sb = pab.tile([128, CC * HW], f16, tag="big")
            nc.sync.dma_start(out=st_sb, in_=std_[:, :])
            hhT = pah.tile([128, HC * C], bf16, tag="hhT")
            hh2T = pah.tile([128, HC * C], bf16, tag="hh2T")

            # A2: HhT[k, c] (+ h_b broadcast along free dim), Hh2T = HhT^2
            for j in range(HC):
                ps = pconv.tile([128, 512], f32, tag="cv")
                for ci in range(CC):
                    nc.tensor.matmul(
                        ps,
                        st_sb[:, ci * HW + j * 128: ci * HW + (j + 1) * 128],
                        hwt[:, ci * C: (ci + 1) * C],
                        start=(ci == 0), stop=False,
                    )
                nc.tensor.matmul(ps, ones16, hbb, start=False, stop=True)
                nc.scalar.activation(out=hhT[:, j * C: (j + 1) * C], in_=ps,
                                     func=ACT.Copy)
                nc.scalar.activation(
                    out=hh2T[:, j * C: (j + 1) * C], in_=hhT[:, j * C: (j + 1) * C],
                    func=ACT.Square,
                )

            # A3: G[c, k]
            gwt = paw.tile([128, CC * C], f16, tag="wconv")
            nc.sync.dma_start(out=gwt, in_=gwtd[:, :])
            gb = paw.tile([128, CC], f32, tag="bias")
            nc.sync.dma_start(out=gb, in_=gbd[:, :])
            sk_sb = pab.tile([128, CC * HW], f16, tag="big")
            nc.sync.dma_start(out=sk_sb, in_=stkd[:, :])
            g_sb = pag.tile([128, CC * HW], f16, tag="gsb")
            for co in range(CC):
                for s0, w in SPANS:
                    ps = pconv.tile([128, 512], f32, tag="cv")
                    for ci in range(CC):
                        nc.tensor.matmul(
                            ps[:, :w],
                            gwt[:, ci * C + co * 128: ci * C + (co + 1) * 128],
                            sk_sb[:, ci * HW + s0: ci * HW + s0 + w],
                            start=(ci == 0), stop=(ci == CC - 1),
                        )
                    nc.scalar.activation(
                        out=g_sb[:, co * HW + s0: co * HW + s0 + w], in_=ps[:, :w],
                        func=ACT.Identity, bias=gb[:, co: co + 1], scale=1.0,
                    )

            # A4: Fq[c, q]
            fwt = paw.tile([128, CC * C], f16, tag="wconv")
            nc.sync.dma_start(out=fwt, in_=fwtd[:, :])
            fb = paw.tile([128, CC], f32, tag="bias")
            nc.sync.dma_start(out=fb, in_=fbd[:, :])
            ck_sb = pag.tile([128, CC * QB], f16, tag="ck")
            nc.sync.dma_start(out=ck_sb, in_=ckd[:, :])
            fq_sb = pag.tile([128, CC * QB], f16, tag="fq")
            for co in range(CC):
                for qh in range(2):
                    ps = pconv.tile([128, 288], f32, tag="cv")
                    for ci in range(CC):
                        nc.tensor.matmul(
                            ps,
                            fwt[:, ci * C + co * 128: ci * C + (co + 1) * 128],
                            ck_sb[:, ci * QB + qh * 288: ci * QB + (qh + 1) * 288],
                            start=(ci == 0), stop=(ci == CC - 1),
                        )
                    nc.scalar.activation(
                        out=fq_sb[:, co * QB + qh * 288: co * QB + (qh + 1) * 288],
                        in_=ps, func=ACT.Identity, bias=fb[:, co: co + 1], scale=1.0,
                    )
            pconv.release()
            paw.release()

            # A5: E = exp(logitsT)  [k, q]
            plog = tc.alloc_tile_pool(name="plog", bufs=3, space="PSUM")
            e_sb = pab.tile([128, HC * QB], bf16, tag="big")
            for j in range(HC):
                for qh in range(2):
                    ps = plog.tile([128, 288], f32, tag="lg")
                    for co in range(CC):
                        nc.tensor.matmul(
                            ps,
                            g_sb[:, co * HW + j * 128: co * HW + (j + 1) * 128],
                            fq_sb[:, co * QB + qh * 288: co * QB + (qh + 1) * 288],
                            start=(co == 0), stop=(co == CC - 1),
                        )
                    nc.scalar.activation(
                        out=e_sb[:, j * QB + qh * 288: j * QB + (qh + 1) * 288],
                        in_=ps, func=ACT.Exp,
                    )
            plog.release()
            pag.release()

            # A6: stats  U = E^T @ [HhT | Hh2T | ones]; normalize; mean/std
            nscr = tc.alloc_tile_pool(name="nscr", bufs=3)
            pstat = tc.alloc_tile_pool(name="pstat", bufs=4, space="PSUM")
            pden = tc.alloc_tile_pool(name="pden", bufs=2, space="PSUM")
            q0 = 0
            for qc, qn in enumerate(QCH):
                u1 = pstat.tile([128, C], f32, tag="u")
                u2 = pstat.tile([128, C], f32, tag="u")
                ud = pden.tile([128, 1], f32, tag="d")
                for j in range(HC):
                    esl = e_sb[:, j * QB + q0: j * QB + q0 + qn]
                    nc.tensor.matmul(u1[:qn], esl, hhT[:, j * C: (j + 1) * C],
                                     start=(j == 0), stop=(j == HC - 1))
                    nc.tensor.matmul(u2[:qn], esl, hh2T[:, j * C: (j + 1) * C],
                                     start=(j == 0), stop=(j == HC - 1))
                    nc.tensor.matmul(ud[:qn], esl, ones_b,
                                     start=(j == 0), stop=(j == HC - 1))
                recip = nscr.tile([128, 1], f32, tag="recip")
                nc.vector.reciprocal(out=recip[:qn], in_=ud[:qn])
                nc.vector.tensor_scalar(
                    out=mst[:qn, qc, 0:512], in0=u1[:qn], scalar1=recip[:qn],
                    scalar2=None, op0=OP.mult,
                )
                e2n = nscr.tile([128, C], f32, tag="e2n")
                nc.vector.tensor_scalar(out=e2n[:qn], in0=u2[:qn], scalar1=recip[:qn],
                                        scalar2=None, op0=OP.mult)
                m2 = nscr.tile([128, C], f32, tag="m2")
                nc.scalar.activation(out=m2[:qn], in_=mst[:qn, qc, 0:512], func=ACT.Square)
                nc.vector.scalar_tensor_tensor(out=e2n[:qn], in0=m2[:qn], scalar=-1.0,
                                               in1=e2n[:qn], op0=OP.mult, op1=OP.add)
                nc.vector.tensor_scalar_max(e2n[:qn], e2n[:qn], 0.0)
                nc.scalar.activation(out=mst[:qn, qc, 512:1024], in_=e2n[:qn], func=ACT.Sqrt)
                q0 += qn
            pden.release()
            pstat.release()
            nscr.release()
            pah.release()
            pab.release()

            if debug:
                nc.sync.dma_start(out=dmstd[:, :], in_=mst.rearrange("p a b -> p (a b)"))

            # A7: AllGather stats within the 4-core batch group
            q0 = 0
            for qc, qn in enumerate(QCH):
                nc.sync.dma_start(out=ag_in[q0: q0 + qn, :], in_=mst[:qn, qc, :])
                q0 += qn
            nc.gpsimd.collective_compute(
                "AllGather", mybir.AluOpType.bypass,
                replica_groups=[[0, 1, 2, 3], [4, 5, 6, 7]],
                ins=[ag_in.ap().opt()],
                outs=[ag_out.ap().opt()],
            )
            pam.release()

            # receive + select this core's 128-channel slice via one-hot wsel
            pb = tc.alloc_tile_pool(name="pb", bufs=1)
            pmid = tc.alloc_tile_pool(name="pmid", bufs=1)
            mean_sl = pmid.tile([128, HC, 128], f16, tag="mean_sl")
            std_sl = pmid.tile([128, HC, 128], f16, tag="std_sl")
            gathv = ag_out.ap().rearrange("(hc p) c -> hc p c", p=128)
            for j in range(HC):
                fc = stream.tile([128, 1024], f16, tag="selc")
                nc.sync.dma_start(out=fc, in_=gathv[j])
                for half, sl in ((0, mean_sl), (1, std_sl)):
                    nc.vector.tensor_scalar(
                        out=sl[:, j, :], in0=fc[:, half * 512: half * 512 + 128],
                        scalar1=wsel[:, 0:1], scalar2=None, op0=OP.mult)
                    for p in range(1, 4):
                        nc.vector.scalar_tensor_tensor(
                            out=sl[:, j, :],
                            in0=fc[:, half * 512 + p * 128: half * 512 + (p + 1) * 128],
                            scalar=wsel[:, p: p + 1], in1=sl[:, j, :],
                            op0=OP.mult, op1=OP.add)

            # A8: contT via PE transpose
            ptr16 = tc.alloc_tile_pool(name="ptr16", bufs=2, space="PSUM")
            cont = pmid.tile([128, HW], f16, tag="cont")
            nc.sync.dma_start(out=cont, in_=contd[:, :])
            contT = pmid.tile([128, HC, 128], f16, tag="contT")
            for j in range(HC):
                pt = ptr16.tile([128, 128], f16, tag="tr16")
                nc.tensor.transpose(pt, cont[:, j * 128: (j + 1) * 128], ident)
                nc.scalar.activation(out=contT[:, j, :], in_=pt, func=ACT.Copy)

            # ================= Phase B: projections + x + D ================
            pspan = tc.alloc_tile_pool(name="pspan", bufs=5, space="PSUM", side="right")
            mean_c = pb.tile([128, HW], f32, tag="mean_c")
            std_c = pb.tile([128, HW], f32, tag="std_c")
            cont_c = pb.tile([128, HW], f32, tag="cont_c")
            for s0, w in SPANS:
                pm = pspan.tile([128, 512], f32, tag="sp")
                psd = pspan.tile([128, 512], f32, tag="sp")
                pcc = pspan.tile([128, 512], f32, tag="sp")
                for j in range(HC):
                    rhs = stream.tile([128, 512], f16, tag="at")
                    nc.sync.dma_start(out=rhs[:, :w], in_=at16d[j, :, s0: s0 + w])
                    st_, sp_ = (j == 0), (j == HC - 1)
                    nc.tensor.matmul(pm[:, :w], mean_sl[:, j, :], rhs[:, :w], start=st_, stop=sp_)
                    nc.tensor.matmul(psd[:, :w], std_sl[:, j, :], rhs[:, :w], start=st_, stop=sp_)
                    nc.tensor.matmul(pcc[:, :w], contT[:, j, :], rhs[:, :w], start=st_, stop=sp_)
                nc.scalar.activation(out=mean_c[:, s0: s0 + w], in_=pm[:, :w], func=ACT.Copy)
                nc.scalar.activation(out=std_c[:, s0: s0 + w], in_=psd[:, :w], func=ACT.Copy)
                nc.vector.tensor_copy(cont_c[:, s0: s0 + w], pcc[:, :w])
            pmid.release()

            # W (lr*WtW) resident: loaded now so the 10.6 MB DMA overlaps B2-B4
            wpool = tc.alloc_tile_pool(name="wpool", bufs=1, side="right")
            w16 = wpool.tile([128, HC * HW], f16, tag="w16")
            nc.sync.dma_start(out=w16, in_=w16d[:, :])

            # B2: group norm (256 groups of 9) -> x (fp16)
            sq = pb.tile([128, HW], f32, tag="sq")
            nc.scalar.activation(out=sq, in_=cont_c, func=ACT.Square)
            s_g = pb.tile([128, GROUPS], f32, tag="sg")
            ss_g = pb.tile([128, GROUPS], f32, tag="ssg")
            cc3 = cont_c.rearrange("p (g d) -> p g d", g=GROUPS)
            nc.vector.tensor_reduce(out=s_g, in_=cc3, axis=AX.X, op=OP.add)
            nc.vector.tensor_reduce(out=ss_g, in_=sq.rearrange("p (g d) -> p g d", g=GROUPS),
                                    axis=AX.X, op=OP.add)
            s2 = pb.tile([128, GROUPS], f32, tag="s2")
            nc.scalar.activation(out=s2, in_=s_g, func=ACT.Square, scale=1.0 / 3.0)
            nc.vector.scalar_tensor_tensor(out=ss_g, in0=s2, scalar=-1.0, in1=ss_g,
                                           op0=OP.mult, op1=OP.add)
            rstd = pb.tile([128, GROUPS], f32, tag="rstd")
            epsc = pb.tile([128, 1], f32, tag="epsc")
            nc.vector.memset(epsc, float(EPS))
            nc.scalar.activation(out=rstd, in_=ss_g, func=ACT.Sqrt,
                                 scale=1.0 / (GS - 1), bias=epsc[:, 0:1])
            nc.vector.reciprocal(out=rstd, in_=rstd)
            mu = pb.tile([128, GROUPS], f32, tag="mu")
            nc.scalar.activation(out=mu, in_=s_g, func=ACT.Copy, scale=1.0 / GS)

            def bc(t):  # [128, G] -> [128, G, GS] broadcast AP (step-0 inner dim)
                ap = t[:, :]
                return bass.AP(tensor=ap.tensor, offset=ap.offset,
                               ap=[list(ap.ap[0]), list(ap.ap[1]), [0, GS]])

            x16 = pb.tile([128, HW], f16, tag="x16")
            sq3 = sq.rearrange("p (g d) -> p g d", g=GROUPS)
            nc.vector.scalar_tensor_tensor(out=sq3, in0=bc(mu), scalar=-1.0, in1=cc3,
                                           op0=OP.mult, op1=OP.add)        # cont - mu
            nc.vector.scalar_tensor_tensor(out=sq3, in0=bc(rstd), scalar=1.0, in1=sq3,
                                           op0=OP.mult, op1=OP.mult)       # * rstd
            nc.vector.scalar_tensor_tensor(out=sq, in0=sq, scalar=1.0, in1=std_c,
                                           op0=OP.mult, op1=OP.mult)       # * std_c
            nc.vector.scalar_tensor_tensor(out=x16, in0=sq, scalar=1.0, in1=mean_c,
                                           op0=OP.mult, op1=OP.add)        # + mean_c
            if debug:
                nc.sync.dma_start(out=dxd[:, :], in_=x16)

            # B3: xT
            xT = pb.tile([128, HC, 128], f16, tag="xT")
            for j in range(HC):
                pt = ptr16.tile([128, 128], f16, tag="tr16")
                nc.tensor.transpose(pt, x16[:, j * 128: (j + 1) * 128], ident)
                nc.scalar.activation(out=xT[:, j, :], in_=pt, func=ACT.Copy)

            # B4: D = x @ (lr*A)
            D = pc_pool.tile([128, HW], f32, tag="D")
            for s0, w in SPANS:
                pd = pspan.tile([128, 512], f32, tag="sp")
                for j in range(HC):
                    rhs = stream.tile([128, 512], f16, tag="at")
                    nc.sync.dma_start(out=rhs[:, :w], in_=alr16d[j, :, s0: s0 + w])
                    nc.tensor.matmul(pd[:, :w], xT[:, j, :], rhs[:, :w],
                                     start=(j == 0), stop=(j == HC - 1))
                nc.vector.tensor_copy(D[:, s0: s0 + w], pd[:, :w])
            if debug:
                nc.sync.dma_start(out=dDd[:, :], in_=D)
            pb.release()
            ptr16.release()

            # ================= Phase C: FISTA ==============================
            # Per iteration: 90 matmuls (5 psum spans x 18 h-chunks) drain
            # into a full-width scratch (1 DVE op per span), then the
            # softshrink/momentum chain runs at 1152-wide half-iterations
            # (fewer, bigger DVE ops), with the y->Y16 PE transposes issued
            # per half so the next iteration's matmuls start early.
            ptr32 = tc.alloc_tile_pool(name="ptr32", bufs=3, space="PSUM")
            fscr = tc.alloc_tile_pool(name="fscr", bufs=3)
            y_rh = pc_pool.tile([128, HW], f32, tag="y_rh")
            Z = [pc_pool.tile([128, HW], f16, tag=f"z{i}", name=f"z{i}r{_r}") for i in range(2)]
            Y16 = [pc_pool.tile([128, HC, 128], f16, tag=f"y16_{i}", name=f"y16_{i}r{_r}")
                   for i in range(2)]
            HALves = [(0, 1152), (1152, 1152)]

            def do_transposes_half(k, h0):
                for j in range(h0 // 128, (h0 + 1152) // 128):
                    pt = ptr32.tile([128, 128], f32, tag="tr32")
                    nc.tensor.transpose(pt, y_rh[:, j * 128: (j + 1) * 128], ident32)
                    nc.scalar.activation(out=Y16[k % 2][:, j, :], in_=pt, func=ACT.Copy)

            # iteration 1: y0 = 0 so arg = D
            for h0, hw_ in HALves:
                hs = slice(h0, h0 + hw_)
                ctmp = fscr.tile([128, 1152], f32, tag="ctmp")
                nc.vector.tensor_scalar(out=ctmp, in0=D[:, hs],
                                        scalar1=-lam, scalar2=lam, op0=OP.max, op1=OP.min)
                nc.vector.scalar_tensor_tensor(out=Z[1][:, hs], in0=ctmp, scalar=-1.0,
                                               in1=D[:, hs], op0=OP.mult, op1=OP.add)
                nc.vector.tensor_copy(y_rh[:, hs], Z[1][:, hs])
                do_transposes_half(1, h0)

            for k in range(2, n_iter + 1):
                beta = betas[k]
                z_prev, z_next = Z[(k - 1) % 2], Z[k % 2]
                y_lhs = Y16[(k - 1) % 2]
                targ = fscr.tile([128, HW], f32, tag="targ")
                for si, (s0, w) in enumerate(SPANS):
                    ps = pspan.tile([128, 512], f32, tag="sp")
                    for j in range(HC):
                        nc.tensor.matmul(ps[:, :w], y_lhs[:, j, :],
                                         w16[:, j * HW + s0: j * HW + s0 + w],
                                         start=(j == 0), stop=(j == HC - 1))
                    # targ[:, span] = y - psum  (evacuates the psum bank)
                    nc.vector.scalar_tensor_tensor(out=targ[:, s0: s0 + w], in0=ps[:, :w],
                                                   scalar=-1.0, in1=y_rh[:, s0: s0 + w],
                                                   op0=OP.mult, op1=OP.add)
                    # after spans {0,1,2} -> half 0; after {3,4} -> half 1
                    if si not in (2, 4):
                        continue
                    h0, hw_ = HALves[0] if si == 2 else HALves[1]
                    hs = slice(h0, h0 + hw_)
                    nc.vector.scalar_tensor_tensor(out=targ[:, hs], in0=targ[:, hs],
                                                   scalar=1.0, in1=D[:, hs],
                                                   op0=OP.mult, op1=OP.add)
                    ctmp = fscr.tile([128, 1152], f32, tag="ctmp")
                    nc.vector.tensor_scalar(out=ctmp, in0=targ[:, hs],
                                            scalar1=-lam, scalar2=lam,
                                            op0=OP.max, op1=OP.min)
                    nc.vector.scalar_tensor_tensor(out=z_next[:, hs], in0=ctmp,
                                                   scalar=-1.0, in1=targ[:, hs],
                                                   op0=OP.mult, op1=OP.add)
                    if k < n_iter:
                        d_ = fscr.tile([128, 1152], f32, tag="d_")
                        nc.vector.scalar_tensor_tensor(out=d_, in0=z_prev[:, hs],
                                                       scalar=-1.0, in1=z_next[:, hs],
                                                       op0=OP.mult, op1=OP.add)
                        nc.vector.scalar_tensor_tensor(out=y_rh[:, hs], in0=d_,
                                                       scalar=float(beta),
                                                       in1=z_next[:, hs],
                                                       op0=OP.mult, op1=OP.add)
                        do_transposes_half(k, h0)

            zf = Z[n_iter % 2]
            out_sb = pc_pool.tile([128, HW], f32, tag="y_rh", name=f"out_sb_r{_r}")
            nc.vector.tensor_copy(out_sb, zf)
            nc.sync.dma_start(out=zoutd[:, :], in_=out_sb)

            ptr32.release()
            pspan.release()
            wpool.release()
            stream.release()
            fscr.release()
            pc_pool.release()
            p0.release()


    return nc


def _host_prep(inputs):
    A = np.asarray(inputs["A"], np.float32)
    WtW = (A.T @ A).astype(np.float32)
    v = np.full((HW,), 1.0 / math.sqrt(HW), np.float32)
    for _ in range(POWER_ITERS):
        w = (WtW @ v).astype(np.float32)
        v = (w / np.float32(np.linalg.norm(w))).astype(np.float32)
    L = np.float32(v @ (WtW @ v))
    lr = np.float32(1.0 / L)
    lam = float(np.float32(ALPHA * lr))

    # betas[k] = (t_k - 1) / t_{k+1} used by iteration k (1-indexed)
    betas = [0.0, 0.0]  # index 0 unused; beta_1 = 0
    t = 1.0
    for _ in range(2, MAXITER + 1):
        tn = (1.0 + math.sqrt(1.0 + 4.0 * t * t)) / 2.0
        t2 = (1.0 + math.sqrt(1.0 + 4.0 * tn * tn)) / 2.0
        betas.append((tn - 1.0) / t2)
        t = tn
    # recompute properly: t_1 = 1; beta_k = (t_k - 1)/t_{k+1}
    ts = [1.0]
    for _ in range(MAXITER):
        ts.append((1.0 + math.sqrt(1.0 + 4.0 * ts[-1] ** 2)) / 2.0)
    betas = [0.0] + [(ts[k - 1] - 1.0) / ts[k] for k in range(1, MAXITER + 1)]

    chunk16 = lambda X: np.ascontiguousarray(X.reshape(HC, 128, HW)).astype(np.float16)
    W = (lr * WtW).astype(np.float32)
    w16 = np.ascontiguousarray(
        W.reshape(HC, 128, HW).transpose(1, 0, 2).reshape(128, HC * HW)
    ).astype(np.float16)
    at16 = chunk16(np.ascontiguousarray(A.T))
    alr16 = chunk16(lr * A)

    pm = lambda X, f: np.ascontiguousarray(     # [N*128, f] -> [128, N*f]
        X.reshape(-1, 128, f).transpose(1, 0, 2).reshape(128, -1))

    fwt = pm(np.asarray(inputs["f_w"], np.float32).T.copy(), C).astype(np.float16)
    gwt = pm(np.asarray(inputs["g_w"], np.float32).T.copy(), C).astype(np.float16)
    hwt = pm(np.asarray(inputs["h_w"], np.float32).T.copy(), C).astype(np.float16)
    fb = np.ascontiguousarray(np.asarray(inputs["f_b"], np.float32).reshape(CC, 128).T)
    gb = np.ascontiguousarray(np.asarray(inputs["g_b"], np.float32).reshape(CC, 128).T)
    hbb = np.asarray(inputs["h_b"], np.float32)[None, :].astype(np.float16)

    in_maps = []
    for i in range(NCORES):
        b, q, cs = i // 4, i % 4, i % 4
        stk = pm(np.asarray(inputs["style_key"], np.float32)[b].reshape(C, HW), HW).astype(np.float16)
        st = pm(np.asarray(inputs["style"], np.float32)[b].reshape(C, HW), HW).astype(np.float16)
        ck = pm(np.asarray(inputs["content_key"], np.float32)[b].reshape(C, HW)[:, q * QB:(q + 1) * QB].copy(), QB).astype(np.float16)
        cont = np.ascontiguousarray(
            np.asarray(inputs["content"], np.float32)[b].reshape(C, HW)[cs * 128:(cs + 1) * 128, :]
        ).astype(np.float16)
        wsel = np.zeros((128, 4), np.float32)
        wsel[:, cs] = 1.0
        in_maps.append({
            "w16": w16, "at16": at16, "alr16": alr16,
            "stk": stk, "st": st, "ck": ck, "cont": cont,
            "fwt": fwt, "gwt": gwt, "hwt": hwt,
            "fb": fb, "gb": gb, "hbb": hbb, "wsel": wsel,
        })
    return lam, betas, in_maps


def kernel(content, style, content_key, style_key,
           f_w, f_b, g_w, g_b, h_w, h_b, A,
           _n_iter=MAXITER, _debug=False, _trace=False, _reps=1):
    from concourse.bass_utils import run_bass_kernel_spmd

    inputs = dict(content=content, style=style, content_key=content_key,
                  style_key=style_key, f_w=f_w, f_b=f_b, g_w=g_w, g_b=g_b,
                  h_w=h_w, h_b=h_b, A=A)
    lam, betas, in_maps = _host_prep(inputs)

    key = (float(lam), _n_iter, _debug, _reps)
    if key not in _CACHE:
        _CACHE[key] = _build_graph(lam, betas, _n_iter, debug=_debug, reps=_reps)
    nc = _CACHE[key]

    res = run_bass_kernel_spmd(nc, in_maps, core_ids=list(range(NCORES)),
                               trace=_trace)
    out = np.zeros((B, C, HW), np.float32)
    for i in range(NCORES):
        b, cs = i // 4, i % 4
        out[b, cs * 128:(cs + 1) * 128, :] = res.results[i]["zout"]
    kernel._last = res
    return out.reshape(B, C, Hs, Ws)

